# revision 9
# baseline (speedup 1.0000x reference)
"""Trainium2 Bass kernel for nn_HeatEquation1D.

The reference applies a fixed 62x62 Crank-Nicolson step matrix 100 times to
u0[:, 1:-1] via lax.scan, then zero-pads the boundary columns.  Algebraically
that whole scan is a single matmul:

    out = u0 @ W64,   W64[1:63, 1:63] = (step_matrix^100).T,  zero elsewhere

(the zero rows/cols of W64 implement both the dropped boundary inputs and the
zero Dirichlet boundary outputs).  W64 is computed on the host in float64.

Device kernel (per core, pure data parallel over 8 cores):
  - u shard (65536, 64) f32 is processed in 16 groups of 4096 rows; each
    group is one contiguous 1 MiB DMA ([128, 32, 64]: partition p holds 32
    consecutive rows, 8 KiB contiguous per partition).
  - Per 1024-row block within a group, for each 128-column chunk c
    (2 rows/partition), PE transpose:
      T1[:, c] = X[:, 2c:2c+2, :].T   (128, 128), into one PSUM bank tile.
  - One DVE copy PSUM -> SBUF casts T1 to bf16 (matmul then runs at 1
    cycle/row instead of fp32's 4).
  - matmul with the *transposed chunk as stationary* (bf16) and a host-built
    bf16 BD = block_diag(W64, W64) (128x128) as the moving operand:
      Y[:, c] = T1s[:, c].T @ BD
    BD's block-diagonal structure applies W64 to each row of the pair and the
    result lands *batch-major* in PSUM (fp32) -- no second transpose needed.
  - One ScalarE (ACT) copy PSUM -> SBUF per block (keeps VectorE free for the
    T1 casts), then one contiguous 1 MiB DMA out per group.

Per-core traffic: 2 x 16.78 MB ~= 94 us at ~358 GB/s; PE/DVE/ACT work all
fit under that, so the kernel is memory-bound as intended.
"""

import numpy as np

BATCH = 524288
NX = 64
N_INNER = NX - 2
NUM_STEPS = 100
N_CORES = 8
ROWS_PER_CORE = BATCH // N_CORES          # 65536
P = 128
ROWS_PER_PART = 8                          # rows per partition per block
ROWS_PER_BLOCK = P * ROWS_PER_PART         # 1024
N_BLOCKS = ROWS_PER_CORE // ROWS_PER_BLOCK  # 64
CHUNKS = (ROWS_PER_PART * NX) // P         # 4 chunks of 128 columns
BLK_F = ROWS_PER_PART * NX                 # 512 free elems per block
G = 4                                      # blocks per DMA group (1 MiB)
N_GROUPS = N_BLOCKS // G                   # 16
GRP_F = G * BLK_F                          # 2048 free elems per group

# Set by callers that want a profile; results object stashed in LAST_RESULTS.
TRACE = False
LAST_RESULTS = None

_NC_CACHE = {}


def _build_nc(reps=1, dma_only=False, groups=None, act_out_dma=True, bf16_t1p=False):
    from concourse import bacc, mybir
    from concourse.tile import TileContext

    if groups is None:
        groups = [G] * N_GROUPS
    assert sum(groups) == N_BLOCKS, groups

    nc = bacc.Bacc("TRN2", target_bir_lowering=False, debug=False)
    f32 = mybir.dt.float32
    bf16 = mybir.dt.bfloat16

    u = nc.dram_tensor("u", [ROWS_PER_CORE, NX], f32, kind="ExternalInput")
    bd_d = nc.dram_tensor("bd", [P, P], bf16, kind="ExternalInput")
    id_d = nc.dram_tensor("ident", [P, P], f32, kind="ExternalInput")
    out = nc.dram_tensor("out", [ROWS_PER_CORE, NX], f32, kind="ExternalOutput")

    # group of k blocks starting at block b0: partition p holds the 8k
    # consecutive rows [b0*1024 + p*8k, +8k), flattened along free dim.
    def grp_view(t, b0, k):
        r = k * ROWS_PER_PART
        return t[b0 * ROWS_PER_BLOCK : (b0 + k) * ROWS_PER_BLOCK].rearrange(
            "(p r) f -> p (r f)", p=P, r=r
        )

    out_eng = nc.scalar if act_out_dma else nc.sync

    max_k = max(groups)
    xbufs = 5 if max_k <= 4 else 3
    ybufs = 4 if max_k <= 4 else 3

    with TileContext(nc) as tc:
        with (
            tc.tile_pool(name="consts", bufs=1) as cpool,
            tc.tile_pool(name="xin", bufs=xbufs) as xpool,
            tc.tile_pool(name="t1s", bufs=3) as tpool,
            tc.tile_pool(name="yout", bufs=ybufs) as ypool,
            tc.tile_pool(name="ps_t", bufs=2, space="PSUM") as pst,
            tc.tile_pool(name="ps_y", bufs=2, space="PSUM") as psy,
        ):
            # first group's input DMA goes ahead of the (tiny) const loads so
            # the big transfer starts as early as possible
            first_x = xpool.tile([P, groups[0] * BLK_F], f32)
            nc.sync.dma_start(out=first_x[:], in_=grp_view(u, 0, groups[0]))

            bd_s = cpool.tile([P, P], bf16)
            id_s = cpool.tile([P, P], f32)
            nc.sync.dma_start(out=bd_s[:], in_=bd_d[:])
            nc.sync.dma_start(out=id_s[:], in_=id_d[:])

            for _rep in range(reps):
                b0 = 0
                for gi, k in enumerate(groups):
                    if _rep == 0 and gi == 0:
                        x = first_x
                    else:
                        x = xpool.tile([P, k * BLK_F], f32)
                        nc.sync.dma_start(out=x[:], in_=grp_view(u, b0, k))

                    if dma_only:
                        nc.sync.dma_start(out=grp_view(out, b0, k), in_=x[:])
                        b0 += k
                        continue

                    y = ypool.tile([P, k * BLK_F], f32)
                    for b in range(k):
                        t1p = pst.tile([P, BLK_F], bf16 if bf16_t1p else f32)
                        for c in range(CHUNKS):
                            nc.tensor.transpose(
                                t1p[:, c * P : (c + 1) * P],
                                x[:, b * BLK_F + c * P : b * BLK_F + (c + 1) * P],
                                id_s[:],
                            )
                        t1s = tpool.tile([P, BLK_F], bf16)
                        nc.vector.tensor_copy(out=t1s[:], in_=t1p[:])

                        yp = psy.tile([P, BLK_F], f32)
                        for c in range(CHUNKS):
                            nc.tensor.matmul(
                                yp[:, c * P : (c + 1) * P],
                                t1s[:, c * P : (c + 1) * P],
                                bd_s[:],
                                start=True,
                                stop=True,
                            )
                        nc.scalar.copy(
                            out=y[:, b * BLK_F : (b + 1) * BLK_F], in_=yp[:]
                        )
                    out_eng.dma_start(out=grp_view(out, b0, k), in_=y[:])
                    b0 += k

    nc.compile()
    return nc


def _host_matrices(step_matrix):
    m = np.asarray(step_matrix, dtype=np.float64)
    w_inner = np.linalg.matrix_power(m, NUM_STEPS).T  # right-multiplier, f64
    w64 = np.zeros((NX, NX), dtype=np.float64)
    w64[1 : NX - 1, 1 : NX - 1] = w_inner
    bd = np.zeros((P, P), dtype=np.float64)
    bd[:NX, :NX] = w64
    bd[NX:, NX:] = w64
    return bd.astype(np.float32)


def prepare_inputs(u0, step_matrix):
    """Host-side prep shared by kernel() and the bench harness: per-core
    input maps with dtypes matching the NEFF's declared tensors."""
    import ml_dtypes

    u0 = np.ascontiguousarray(np.asarray(u0, dtype=np.float32))
    assert u0.shape == (BATCH, NX), u0.shape
    bd = _host_matrices(step_matrix).astype(ml_dtypes.bfloat16)
    ident = np.eye(P, dtype=np.float32)
    shards = np.split(u0, N_CORES, axis=0)
    return [{"u": s, "bd": bd, "ident": ident} for s in shards]


def kernel(u0, step_matrix):
    global LAST_RESULTS
    from concourse.bass_utils import run_bass_kernel_spmd

    if "nc" not in _NC_CACHE:
        _NC_CACHE["nc"] = _build_nc()
    nc = _NC_CACHE["nc"]

    in_maps = prepare_inputs(u0, step_matrix)
    res = run_bass_kernel_spmd(
        nc, in_maps, core_ids=list(range(N_CORES)), trace=TRACE
    )
    LAST_RESULTS = res
    return np.concatenate([r["out"] for r in res.results], axis=0)


# revision 11
# speedup vs baseline: 1.0572x; 1.0572x over previous
"""Trainium2 Bass kernel for nn_HeatEquation1D.

The reference applies a fixed 62x62 Crank-Nicolson step matrix 100 times to
u0[:, 1:-1] via lax.scan, then zero-pads the boundary columns.  Algebraically
that whole scan is a single matmul:

    out = u0 @ W64,   W64[1:63, 1:63] = (step_matrix^100).T,  zero elsewhere

(the zero rows/cols of W64 implement both the dropped boundary inputs and the
zero Dirichlet boundary outputs).  W64 is computed on the host in float64.

Device kernel (per core, pure data parallel over 8 cores):
  - u shard (65536, 64) f32 is processed in 16 groups of 4096 rows; each
    group is one contiguous 1 MiB DMA ([128, 32, 64]: partition p holds 32
    consecutive rows, 8 KiB contiguous per partition).
  - Per 1024-row block within a group, for each 128-column chunk c
    (2 rows/partition), PE transpose:
      T1[:, c] = X[:, 2c:2c+2, :].T   (128, 128), into one PSUM bank tile.
  - One DVE copy PSUM -> SBUF casts T1 to bf16 (matmul then runs at 1
    cycle/row instead of fp32's 4).
  - matmul with the *transposed chunk as stationary* (bf16) and a host-built
    bf16 BD = block_diag(W64, W64) (128x128) as the moving operand:
      Y[:, c] = T1s[:, c].T @ BD
    BD's block-diagonal structure applies W64 to each row of the pair and the
    result lands *batch-major* in PSUM (fp32) -- no second transpose needed.
  - One ScalarE (ACT) copy PSUM -> SBUF per block (keeps VectorE free for the
    T1 casts), then one contiguous 1 MiB DMA out per group.

Per-core traffic: 2 x 16.78 MB ~= 94 us at ~358 GB/s; PE/DVE/ACT work all
fit under that, so the kernel is memory-bound as intended.
"""

import numpy as np

BATCH = 524288
NX = 64
N_INNER = NX - 2
NUM_STEPS = 100
N_CORES = 8
ROWS_PER_CORE = BATCH // N_CORES          # 65536
P = 128
ROWS_PER_PART = 8                          # rows per partition per block
ROWS_PER_BLOCK = P * ROWS_PER_PART         # 1024
N_BLOCKS = ROWS_PER_CORE // ROWS_PER_BLOCK  # 64
CHUNKS = (ROWS_PER_PART * NX) // P         # 4 chunks of 128 columns
BLK_F = ROWS_PER_PART * NX                 # 512 free elems per block
G = 4                                      # blocks per DMA group (1 MiB)
N_GROUPS = N_BLOCKS // G                   # 16
GRP_F = G * BLK_F                          # 2048 free elems per group

# Set by callers that want a profile; results object stashed in LAST_RESULTS.
TRACE = False
LAST_RESULTS = None

_NC_CACHE = {}


def _build_nc(reps=1, dma_only=False, groups=None, act_out_dma=True, bf16_t1p=False):
    from concourse import bacc, mybir
    from concourse.tile import TileContext

    if groups is None:
        groups = [G] * N_GROUPS
    assert sum(groups) == N_BLOCKS, groups

    nc = bacc.Bacc("TRN2", target_bir_lowering=False, debug=False)
    f32 = mybir.dt.float32
    bf16 = mybir.dt.bfloat16

    u = nc.dram_tensor("u", [ROWS_PER_CORE, NX], f32, kind="ExternalInput")
    bd_d = nc.dram_tensor("bd", [P, P], bf16, kind="ExternalInput")
    id_d = nc.dram_tensor("ident", [P, P], f32, kind="ExternalInput")
    out = nc.dram_tensor("out", [ROWS_PER_CORE, NX], f32, kind="ExternalOutput")

    # group of k blocks starting at block b0: partition p holds the 8k
    # consecutive rows [b0*1024 + p*8k, +8k), flattened along free dim.
    def grp_view(t, b0, k):
        r = k * ROWS_PER_PART
        return t[b0 * ROWS_PER_BLOCK : (b0 + k) * ROWS_PER_BLOCK].rearrange(
            "(p r) f -> p (r f)", p=P, r=r
        )

    out_eng = nc.scalar if act_out_dma else nc.sync

    max_k = max(groups)
    xbufs = 5 if max_k <= 4 else 3
    ybufs = 4 if max_k <= 4 else 3

    with TileContext(nc) as tc:
        with (
            tc.tile_pool(name="consts", bufs=1) as cpool,
            tc.tile_pool(name="xin", bufs=xbufs) as xpool,
            tc.tile_pool(name="t1s", bufs=3) as tpool,
            tc.tile_pool(name="yout", bufs=ybufs) as ypool,
            tc.tile_pool(name="ps_t", bufs=2, space="PSUM") as pst,
            tc.tile_pool(name="ps_y", bufs=2, space="PSUM") as psy,
        ):
            # first group's input DMA goes ahead of the (tiny) const loads so
            # the big transfer starts as early as possible
            first_x = xpool.tile([P, groups[0] * BLK_F], f32)
            nc.sync.dma_start(out=first_x[:], in_=grp_view(u, 0, groups[0]))

            bd_s = cpool.tile([P, P], bf16)
            id_s = cpool.tile([P, P], f32)
            nc.sync.dma_start(out=bd_s[:], in_=bd_d[:])
            nc.sync.dma_start(out=id_s[:], in_=id_d[:])

            for _rep in range(reps):
                b0 = 0
                for gi, k in enumerate(groups):
                    if _rep == 0 and gi == 0:
                        x = first_x
                    else:
                        x = xpool.tile([P, k * BLK_F], f32)
                        nc.sync.dma_start(out=x[:], in_=grp_view(u, b0, k))

                    if dma_only:
                        nc.sync.dma_start(out=grp_view(out, b0, k), in_=x[:])
                        b0 += k
                        continue

                    y = ypool.tile([P, k * BLK_F], f32)
                    for b in range(k):
                        t1p = pst.tile([P, BLK_F], bf16 if bf16_t1p else f32)
                        for c in range(CHUNKS):
                            nc.tensor.transpose(
                                t1p[:, c * P : (c + 1) * P],
                                x[:, b * BLK_F + c * P : b * BLK_F + (c + 1) * P],
                                id_s[:],
                            )
                        t1s = tpool.tile([P, BLK_F], bf16)
                        nc.vector.tensor_copy(out=t1s[:], in_=t1p[:])

                        yp = psy.tile([P, BLK_F], f32)
                        for c in range(CHUNKS):
                            nc.tensor.matmul(
                                yp[:, c * P : (c + 1) * P],
                                t1s[:, c * P : (c + 1) * P],
                                bd_s[:],
                                start=True,
                                stop=True,
                            )
                        nc.scalar.copy(
                            out=y[:, b * BLK_F : (b + 1) * BLK_F], in_=yp[:]
                        )
                    out_eng.dma_start(out=grp_view(out, b0, k), in_=y[:])
                    b0 += k

    nc.compile()
    return nc


def _host_matrices(step_matrix):
    m = np.asarray(step_matrix, dtype=np.float64)
    w_inner = np.linalg.matrix_power(m, NUM_STEPS).T  # right-multiplier, f64
    w64 = np.zeros((NX, NX), dtype=np.float64)
    w64[1 : NX - 1, 1 : NX - 1] = w_inner
    bd = np.zeros((P, P), dtype=np.float64)
    bd[:NX, :NX] = w64
    bd[NX:, NX:] = w64
    return bd.astype(np.float32)


def prepare_inputs(u0, step_matrix):
    """Host-side prep shared by kernel() and the bench harness: per-core
    input maps with dtypes matching the NEFF's declared tensors."""
    import ml_dtypes

    u0 = np.ascontiguousarray(np.asarray(u0, dtype=np.float32))
    assert u0.shape == (BATCH, NX), u0.shape
    bd = _host_matrices(step_matrix).astype(ml_dtypes.bfloat16)
    ident = np.eye(P, dtype=np.float32)
    shards = np.split(u0, N_CORES, axis=0)
    return [{"u": s, "bd": bd, "ident": ident} for s in shards]


def kernel(u0, step_matrix):
    global LAST_RESULTS
    from concourse.bass_utils import run_bass_kernel_spmd

    if "nc" not in _NC_CACHE:
        _NC_CACHE["nc"] = _build_nc()
    nc = _NC_CACHE["nc"]

    in_maps = prepare_inputs(u0, step_matrix)
    res = run_bass_kernel_spmd(
        nc, in_maps, core_ids=list(range(N_CORES)), trace=TRACE
    )
    LAST_RESULTS = res
    return np.concatenate([r["out"] for r in res.results], axis=0)
